# revision 38
# baseline (speedup 1.0000x reference)
"""Decoder-only attention block (QKV proj + MHA + out proj) on 8 TRN2 cores.

Sharding: core c -> (batch b = c//4, head-group g = c%4). Tensor-parallel over
heads (4 of 16 heads per core), data-parallel over batch (2). Each core
computes a partial c_proj over its 512 input features; host reduces the 4
partials per batch and adds biases.

Schedule notes:
- Phase 1 is token-block-major: QK chains for block tb only need block tb's
  x columns, so the PE starts once ~2.6MB of input has landed instead of
  ~9MB. The V projection is interleaved per block; input DMA issue is spread
  across the sync/gpsimd/vector queues (DMA issue costs ~0.65us each and is
  serial per queue).
- Phase 2 runs a skew-1 software pipeline: the PE stream interleaves
  scores(head i+1) pairs with A@V(head i) pairs at matmul granularity, so
  the scalar-engine exp latency and the gpsimd softmax-denominator reduce
  never stall the PE. Any PE idle gap > ~3.4us would HAM-throttle the array
  to half clock; the pipeline is built to avoid them.
- c_proj for block k is emitted right after its 4 heads finish (PSUM tags
  are shared across phases: qk-acc/proj, v/psot); partials ship as f16.

Self-contained: hardcodes B=2, S=2048, D=2048, H=16.
"""

import os

import numpy as np

NPF16 = np.float16

import concourse.bass as bass
import concourse.bacc as bacc
import concourse.tile as tile
from concourse import mybir
from concourse import library_config
import concourse.bass_utils as bass_utils
import concourse.bass_isa as bass_isa
from concourse.bass_interp import get_hw_module

B, S, D = 2, 2048, 2048
H, DH = 16, 128
N_CORES = 8
HL = H // 4            # 4 heads per core
FL = HL * DH           # 512 local features per core
KT = D // 128          # 16 contraction tiles
TT = S // 128          # 16 token tiles
QB = S // 512          # 4 token blocks
SCALE = 1.0 / float(np.sqrt(DH))
FW = KT * 128          # 2048 cols per f-chunk of packed wqk

F16 = mybir.dt.float16
F32 = mybir.dt.float32

# Stash of the last BassKernelResults (for the local test harness only).
LAST_RESULTS = None
_PROG_CACHE = {}


def _build_program(use_mask):

    nc = bacc.Bacc("TRN2", target_bir_lowering=False, debug=False,
                   num_devices=N_CORES)

    # Host-packed layouts (see kernel() for the packing):
    #   xt  [128, (tb, kt, 512)]  x^T, token-block-major
    #   wqk [128, (f, kt, 128)]   q|k weight, f-chunk-major
    #   wv  [128, (kt, 512)]
    #   wp  [FL, D]
    xt_d = nc.dram_tensor("xt", [128, QB * KT * 512], F16, kind="ExternalInput")
    wqk_d = nc.dram_tensor("wqk", [128, 8 * FW], F16, kind="ExternalInput")
    wv_d = nc.dram_tensor("wv", [128, KT * FL], F16, kind="ExternalInput")
    wp_d = nc.dram_tensor("wp", [FL, D], F16, kind="ExternalInput")
    bqk_d = nc.dram_tensor("bqk", [128, 8], F32, kind="ExternalInput")
    kb_d = nc.dram_tensor("kb", [128, KT], F32, kind="ExternalInput")
    out_d = nc.dram_tensor("out", [S, D], F16, kind="ExternalOutput")

    xt_ap, wqk_ap, wv_ap, wp_ap = xt_d.ap(), wqk_d.ap(), wv_d.ap(), wp_d.ap()
    bqk_ap, kb_ap, out_ap = bqk_d.ap(), kb_d.ap(), out_d.ap()

    with (
        tile.TileContext(nc) as tc,
        tc.tile_pool(name="pers", bufs=1) as pers,
        tc.tile_pool(name="pwv", bufs=1) as pwv,
        tc.tile_pool(name="pxB", bufs=1) as pxB,
        tc.tile_pool(name="ps", bufs=1, space="PSUM") as ps,
    ):
        # ---- persistent tiles ----
        qt = [pers.tile([128, S], F16, tag=f"qt{h}", name=f"qt{h}") for h in range(HL)]
        ktt = [pers.tile([128, S], F16, tag=f"kt{h}", name=f"ktt{h}") for h in range(HL)]
        ot = [pers.tile([128, S], F16, tag=f"ot{h}", name=f"ot{h}") for h in range(HL)]
        vaug = [[pers.tile([128, DH], F16, tag=f"v{t}_{h}", name=f"v{t}_{h}")
                 for h in range(HL)] for t in range(TT)]
        wp_sb = [pers.tile([128, D], F16, tag=f"wp{h}", name=f"wp{h}")
                 for h in range(HL)]
        bqk_sb = pers.tile([128, 8], F32, tag="bqk", name="bqk_sb")
        kb_sb = pers.tile([128, KT], F32, tag="kb", name="kb_sb")

        nc.scalar.dma_start(bqk_sb[:], bqk_ap[:])
        if use_mask:
            nc.scalar.dma_start(kb_sb[:], kb_ap[:])

        wv_sb = pwv.tile([128, KT * FL], F16, tag="wv", name="wv_sb")
        xt3 = pxB.tile([128, KT * 512], F16, tag="xt3", name="xt3")

        state = {}

        def gen_scores(qb, h, pool):
            """scores^T -> exp -> E tiles, denominator tree + gpsimd + rcp.

            Yields 8 times (one per k-tile pair) for PE-stream interleaving.
            """
            e_tiles = []
            l1 = {}
            l2 = {}
            for p in range(KT // 2):
                pss = ps.tile([128, 1024], F32, tag="pss", bufs=2, name="pss")
                for half in range(2):
                    kt = 2 * p + half
                    nc.tensor.matmul(
                        pss[:, half * 512:(half + 1) * 512],
                        ktt[h][:, kt * 128:(kt + 1) * 128],
                        qt[h][:, qb * 512:(qb + 1) * 512],
                        start=True, stop=True,
                    )
                e = pool.tile([128, 1024], F16, tag=f"e{p}", bufs=3,
                              name=f"e{p}")
                nc.scalar.activation(
                    e[:], pss[:], mybir.ActivationFunctionType.Exp,
                    scale=SCALE,
                )
                if use_mask:
                    for half in range(2):
                        kt = 2 * p + half
                        sl = e[:, half * 512:(half + 1) * 512]
                        nc.vector.tensor_scalar_mul(sl, sl, kb_sb[:, kt:kt + 1])
                e_tiles.append(e)
                # denominator tree, spread so no round emits more than one
                # add: the c_proj stage CASTs share the vector queue and
                # gate PSUM-bank reuse ~2 pieces later, while the tree has
                # 1-2 slots of slack before the gpsimd reduce needs it.
                # The tree TAIL (l2b/dnb/dn + reduce) is emitted by gen_av
                # two slots later, one op per round, so no slot ends with
                # an add burst in front of the next slot's CASTs.
                if p % 2 == 1:
                    i = p // 2
                    t_ = pool.tile([128, 1024], F16, tag=f"l1{i % 2}", bufs=3,
                                   name=f"l1{i % 2}")
                    nc.vector.tensor_add(t_[:], e_tiles[p - 1][:],
                                         e_tiles[p][:])
                    l1[i] = t_
                if p == 4:
                    t_ = pool.tile([128, 1024], F16, tag="l2a", bufs=2,
                                   name="l2a")
                    nc.vector.tensor_add(t_[:], l1[0][:], l1[1][:])
                    l2[0] = t_
                if p == 6:
                    t_ = pool.tile([128, 512], F16, tag="dna", bufs=2,
                                   name="dna")
                    nc.vector.tensor_add(t_[:], l2[0][:, 0:512],
                                         l2[0][:, 512:1024])
                    l2["dna"] = t_
                yield
            state[(qb, h)] = (e_tiles, l1, l2["dna"])

        def gen_av(qb, h, pool):
            """O^T = V^T A^T, evicted UNNORMALIZED to a scratch tile so the
            PSUM bank never waits on the gpsimd denominator chain. The
            normalization multiply happens in finish_av once the reciprocal
            is ready (1 slot of slack). Also finishes this head's
            denominator tree (one op per round) + the gpsimd reduce.
            Yields 8 times."""
            e_tiles, l1, dna = state[(qb, h)]
            dnr = None
            psot = ps.tile([128, 512], F32, tag="pv", bufs=2, name="psot")
            for p in range(KT // 2):
                for half in range(2):
                    kt = 2 * p + half
                    nc.tensor.matmul(
                        psot[:],
                        vaug[kt][h][:],
                        e_tiles[p][:, half * 512:(half + 1) * 512],
                        start=(kt == 0), stop=(kt == KT - 1),
                    )
                if p == 0:
                    l2b = pool.tile([128, 1024], F16, tag="l2b", bufs=2,
                                    name="l2b")
                    nc.vector.tensor_add(l2b[:], l1[2][:], l1[3][:])
                elif p == 1:
                    dnb = pool.tile([128, 512], F16, tag="dnb", bufs=2,
                                    name="dnb")
                    nc.vector.tensor_add(dnb[:], l2b[:, 0:512],
                                         l2b[:, 512:1024])
                elif p == 2:
                    dn = pool.tile([128, 512], F16, tag="dn", bufs=2,
                                   name="dn")
                    nc.vector.tensor_add(dn[:], dna[:], dnb[:])
                elif p == 3:
                    dnr = pool.tile([128, 512], F32, tag="dnr", bufs=2,
                                    name="dnr")
                    nc.gpsimd.partition_all_reduce(
                        dnr[:], dn[:], channels=128,
                        reduce_op=bass_isa.ReduceOp.add)
                yield
            os_ = pool.tile([128, 512], F16, tag="osc", bufs=2, name="osc")
            nc.scalar.copy(os_[:], psot[:])
            # reciprocal here (end of slot i+2) rather than in finish_av:
            # the reduce landed mid-slot, and this leaves only the mul in
            # front of the next slot's first c_proj stage CAST.
            rcp = pool.tile([128, 512], F32, tag="rcp", bufs=1, name="rcp")
            nc.vector.reciprocal_approx_fast(rcp[:], dnr[:])
            state[(qb, h)] = (rcp, os_)

        def finish_av(qb, h, pool):
            rcp, os_ = state.pop((qb, h))
            nc.vector.tensor_mul(ot[h][:, qb * 512:(qb + 1) * 512],
                                 os_[:], rcp[:])

        def gen_proj(k, pool, tail=False):
            """c_proj for token block k. Yields 32 times (2 matmuls each).

            tail=True ships per-chain [128,512] f16 DMAs alternating the
            sync/gpsimd issue queues so the final flush spreads across more
            DMA engines."""
            for ti, t in enumerate(range(k * 4, k * 4 + 4)):
                for nbp in range(2):
                    st = (None if tail else
                          pool.tile([128, 1024], F16, tag="stage", bufs=2,
                                    name="stage"))
                    for half in range(2):
                        nb = 2 * nbp + half
                        psp = ps.tile([128, 512], F32, tag="qk", bufs=2,
                                      name="psp")
                        for h in range(HL):
                            nc.tensor.matmul(
                                psp[:],
                                ot[h][:, t * 128:(t + 1) * 128],
                                wp_sb[h][:, nb * 512:(nb + 1) * 512],
                                start=(h == 0), stop=(h == HL - 1),
                                skip_group_check=True,
                            )
                            if h == 1:
                                yield
                        if tail:
                            s5 = pool.tile([128, 512], F16, tag="stg5",
                                           bufs=2, name="stg5")
                            nc.vector.tensor_copy(s5[:], psp[:])
                            eng = nc.sync if (nb + ti) % 2 == 0 else nc.gpsimd
                            eng.dma_start(
                                out_ap[t * 128:(t + 1) * 128,
                                       nb * 512:(nb + 1) * 512], s5[:])
                        else:
                            nc.vector.tensor_copy(
                                st[:, half * 512:(half + 1) * 512], psp[:])
                            if half == 1:
                                nc.sync.dma_start(
                                    out_ap[t * 128:(t + 1) * 128,
                                           nbp * 1024:(nbp + 1) * 1024],
                                    st[:])
                        yield

        def run_slot(sg, ag, xgs=(), pre=None):
            """One pipeline slot: interleave scores-pairs (sg), A@V-pairs
            (ag) and extra PE work (xgs: (generator, count) pairs — V
            chains, c_proj pieces, tail-proj partials) at matmul
            granularity. `pre` emits the reciprocal+normalize of the head
            finished 3 slots ago BEFORE any c_proj piece of this slot can
            read its ot range."""
            if pre is not None:
                pre()
            for _ in range(KT // 2):
                # xgs (c_proj pieces) go FIRST: their PSUM-evicting CASTs
                # then precede the round's denominator-tree adds in the
                # vector queue. The tree has ~2 slots of slack; the CASTs
                # gate PSUM-bank reuse two pieces later.
                for g, cnt in xgs:
                    for _ in range(cnt):
                        next(g, None)
                if sg is not None:
                    next(sg, None)
                if ag is not None:
                    next(ag, None)
            if ag is not None:
                for _ in ag:
                    pass
            if sg is not None:
                for _ in sg:
                    pass
            # xgs are NOT drained: a c_proj generator spans 4 slots (8 of
            # its 32 pieces per slot); exhaust a generator here only when
            # the caller knows it ends this slot (V chains).

        def emit_qk_chain(f, src, interleave=None):
            acc = ps.tile([128, 512], F32, tag="qk", bufs=2, name="qkacc")
            for kt in range(KT):
                nc.tensor.matmul(
                    acc[:],
                    wqk_sb[:, kt * 1024 + f * 128:kt * 1024 + (f + 1) * 128],
                    src[:, kt * 512:(kt + 1) * 512],
                    start=(kt == 0), stop=(kt == KT - 1),
                    skip_group_check=True,
                )
                if interleave is not None:
                    interleave(kt)
            dest = qt[f] if f < HL else ktt[f - HL]
            return acc, dest

        def emit_v_chain(t, src):
            psv = ps.tile([128, FL], F32, tag="pv", bufs=2, name="psv")
            for kt in range(KT):
                nc.tensor.matmul(
                    psv[:],
                    src[:, kt * 512 + (t % 4) * 128:
                        kt * 512 + (t % 4 + 1) * 128],
                    wv_sb[:, kt * FL:(kt + 1) * FL],
                    start=(kt == 0), stop=(kt == KT - 1),
                )
            for h in range(HL):
                nc.vector.tensor_copy(vaug[t][h][:],
                                      psv[:, h * 128:(h + 1) * 128])

        def gen_v2(t0, t1):
            """Two V chains for the pipeline ramp slots; 16 yields of 2
            matmuls (the trailing vaug copies are emitted before the final
            yield of each chain so 16 next() calls consume everything)."""
            for t in (t0, t1):
                psv = ps.tile([128, FL], F32, tag="pv", bufs=2, name="psv")
                for p in range(KT // 2):
                    for half in range(2):
                        kt = 2 * p + half
                        nc.tensor.matmul(
                            psv[:],
                            xt3[:, kt * 512 + (t % 4) * 128:
                                kt * 512 + (t % 4 + 1) * 128],
                            wv_sb[:, kt * FL:(kt + 1) * FL],
                            start=(kt == 0), stop=(kt == KT - 1),
                        )
                    if p == KT // 2 - 1:
                        for h in range(HL):
                            nc.vector.tensor_copy(vaug[t][h][:],
                                                  psv[:, h * 128:(h + 1) * 128])
                    yield

        # ================= phase 1: QKV projection, token-block-major =====
        with tc.tile_pool(name="pwqk", bufs=1) as pwqk:
            wqk_sb = pwqk.tile([128, 8 * FW], F16, tag="wqk", name="wqk_sb")

            with tc.tile_pool(name="pxA", bufs=1) as pxA:
                xts = [pxA.tile([128, KT * 512], F16, tag="xt0", name="xt0"),
                       pxA.tile([128, KT * 512], F16, tag="xt1", name="xt1")]

                # --- input DMA issues, spread across engine queues ---
                # wqk kt-major pieces matching the interleaved tb0 round
                # order (round kt consumes piece kt). Piece kt0 gates the
                # very first matmul: issue it FIRST on sync, in halves, so
                # the PE starts ~2us sooner; gpsimd carries kt1..15.
                nc.sync.dma_start(wqk_sb[:, 0:256], wqk_ap[:, 0:256])
                nc.sync.dma_start(xts[0][:, 0:512], xt_ap[:, 0:512])
                nc.sync.dma_start(wqk_sb[:, 256:640], wqk_ap[:, 256:640])
                nc.sync.dma_start(wqk_sb[:, 640:1024], wqk_ap[:, 640:1024])
                for kt in range(1, KT):
                    c0 = kt * 1024
                    nc.gpsimd.dma_start(wqk_sb[:, c0:c0 + 1024],
                                        wqk_ap[:, c0:c0 + 1024])
                # Preload the gpsimd ucode lib that partition_all_reduce
                # needs. Lazily it would load at phase-2 start (~7us on the
                # gpsimd queue), delaying the first softmax denominators and
                # stalling the PE. Here it fills gpsimd's idle window after
                # its DMA issues.
                nc.gpsimd.load_library(library_config.attn)
                # xt block 0 (granular, paces the first chains): split the
                # issue cost across the sync and scalar queues so all 16
                # slices land ~5us earlier; block 1 on sync
                for kt in range(1, KT // 2):
                    nc.sync.dma_start(
                        xts[0][:, kt * 512:(kt + 1) * 512],
                        xt_ap[:, kt * 512:(kt + 1) * 512])
                for kt in range(KT // 2, KT):
                    nc.scalar.dma_start(
                        xts[0][:, kt * 512:(kt + 1) * 512],
                        xt_ap[:, kt * 512:(kt + 1) * 512])
                for q in range(8):
                    b1 = KT * 512
                    nc.sync.dma_start(
                        xts[1][:, q * 1024:(q + 1) * 1024],
                        xt_ap[:, b1 + q * 1024:b1 + (q + 1) * 1024])
                # scalar (Activation) queue: wv. (wp is needed only in
                # phase 2 — issued at tb==1 to keep it out of the
                # BW-saturated ramp window.)
                for q in range(8):
                    nc.scalar.dma_start(wv_sb[:, q * 1024:(q + 1) * 1024],
                                        wv_ap[:, q * 1024:(q + 1) * 1024])

                # --- token block 0: all 8 QK chains interleaved at kt
                # granularity. Each early DMA byte feeds 8 chains, so the
                # PE reads ~225GB/s sustained instead of bursting 765GB/s
                # per chain against the ~350GB/s HBM ramp. Needs 8 open
                # PSUM groups: borrow every phase-2 tag (idle here).
                pA = ps.tile([128, 1024], F32, tag="pss", bufs=2, name="pA")
                pB = ps.tile([128, 1024], F32, tag="pss", bufs=2, name="pB")
                pC = ps.tile([128, 512], F32, tag="pv", bufs=2, name="pC")
                pD = ps.tile([128, 512], F32, tag="pv", bufs=2, name="pD")
                pE = ps.tile([128, 512], F32, tag="qk", bufs=2, name="pE")
                pF = ps.tile([128, 512], F32, tag="qk", bufs=2, name="pF")
                accs = [pA[:, 0:512], pA[:, 512:1024],
                        pB[:, 0:512], pB[:, 512:1024],
                        pC[:], pD[:], pE[:], pF[:]]
                for kt in range(KT):
                    for f in range(8):
                        nc.tensor.matmul(
                            accs[f],
                            wqk_sb[:, kt * 1024 + f * 128:
                                   kt * 1024 + (f + 1) * 128],
                            xts[0][:, kt * 512:(kt + 1) * 512],
                            start=(kt == 0), stop=(kt == KT - 1),
                            skip_group_check=True,
                        )
                # evict f=6,7 first: their qk-tag banks gate tb1's first
                # two chains.
                for f in (6, 7, 0, 1, 2, 3, 4, 5):
                    dest = qt[f] if f < HL else ktt[f - HL]
                    nc.vector.tensor_scalar_add(dest[:, 0:512], accs[f],
                                                bqk_sb[:, f:f + 1])
                for t in range(4):
                    emit_v_chain(t, xts[0])
                # prefetch block 2 into the rotated xt0 buffer
                xts.append(pxA.tile([128, KT * 512], F16, tag="xt0",
                                    name="xt0"))
                b2 = 2 * KT * 512
                for q in range(8):
                    nc.sync.dma_start(
                        xts[2][:, q * 1024:(q + 1) * 1024],
                        xt_ap[:, b2 + q * 1024:b2 + (q + 1) * 1024])

                # --- token blocks 1..2, chain-major (data is ahead now) ---
                for tb in range(1, 3):
                    src = xts[tb]
                    for f in range(8):
                        acc, dest = emit_qk_chain(f, src)
                        nc.vector.tensor_scalar_add(
                            dest[:, tb * 512:(tb + 1) * 512],
                            acc[:], bqk_sb[:, f:f + 1])
                    for t in range(4 * tb, 4 * tb + 4):
                        emit_v_chain(t, src)
                    if tb == 1:
                        b3 = 3 * KT * 512
                        for q in range(8):
                            nc.sync.dma_start(
                                xt3[:, q * 1024:(q + 1) * 1024],
                                xt_ap[:, b3 + q * 1024:b3 + (q + 1) * 1024])
                        for h in range(HL):
                            nc.scalar.dma_start(
                                wp_sb[h][:], wp_ap[h * 128:(h + 1) * 128, :])
            # pxA closed (blocks 0-2 consumed)

            # --- block 3: QK only; its V chains move into the phase-2
            # ramp slots so the pipeline starts with full 3-lane units ---
            for f in range(8):
                acc, dest = emit_qk_chain(f, xt3)
                nc.vector.tensor_scalar_add(dest[:, 3 * 512:4 * 512], acc[:],
                                            bqk_sb[:, f:f + 1])
        # pwqk closed

        def gen_p3h01():
            """c_proj(3) partial chains over heads 0-1, staged into the
            dead qt tiles (all scores done by slot 16). 8 pair-units (two
            nb half-chains into one [128,1024] PSUM tile, one CAST), 2
            matmuls per yield, filling the PE time freed by the finished
            scores lane in slots 16-17. The tail then only runs the h=2,3
            half-chains plus an add."""
            for u in range(8):
                ti, pr = divmod(u, 2)
                t = 12 + ti
                pp = ps.tile([128, 1024], F32, tag="pss", bufs=2,
                             name="pp3")
                for half in range(2):
                    nb = 2 * pr + half
                    for hh in (0, 1):
                        nc.tensor.matmul(
                            pp[:, half * 512:(half + 1) * 512],
                            ot[hh][:, t * 128:(t + 1) * 128],
                            wp_sb[hh][:, nb * 512:(nb + 1) * 512],
                            start=(hh == 0), stop=(hh == 1),
                            skip_group_check=True,
                        )
                    if half == 0:
                        yield
                nc.vector.tensor_copy(qt[ti][:, pr * 1024:(pr + 1) * 1024],
                                      pp[:])
                yield

        def gen_tail():
            """c_proj(3) h=2,3 half-chains added onto the staged h=0,1
            partials, shipped per [128,1024] chunk the moment its add
            lands (issue queues alternate so transfers overlap). Runs
            half inside slot 18, half drained after — the adds (1.2us
            each on vector) would oversubscribe a single slot. 2 matmuls
            per yield, 16 yields."""
            for u in range(8):
                ti, pair = divmod(u, 2)
                t = 12 + ti
                s5 = p2.tile([128, 1024], F16, tag="stg5", bufs=3,
                             name="stg5")
                pp = ps.tile([128, 1024], F32, tag="pss", bufs=2,
                             name="pp5")
                for half in range(2):
                    nb = 2 * pair + half
                    for hh in (2, 3):
                        nc.tensor.matmul(
                            pp[:, half * 512:(half + 1) * 512],
                            ot[hh][:, t * 128:(t + 1) * 128],
                            wp_sb[hh][:, nb * 512:(nb + 1) * 512],
                            start=(hh == 2), stop=(hh == 3),
                            skip_group_check=True,
                        )
                    if half == 0:
                        yield
                stg = qt[ti][:, pair * 1024:(pair + 1) * 1024]
                nc.vector.tensor_add(s5[:], stg, pp[:])
                eng = (nc.sync, nc.scalar)[u % 2]
                eng.dma_start(
                    out_ap[t * 128:(t + 1) * 128,
                           pair * 1024:(pair + 1) * 1024], s5[:])
                yield

        # ================= phase 2: skew-2 three-lane pipeline ============
        # slot i: scores(H_i) | A@V(H_{i-2}) | extra (V chains in the two
        # ramp slots, c_proj(k) spread over the four slots after av(k,3)).
        with tc.tile_pool(name="p2", bufs=1) as p2:
            heads = [(k, h) for k in range(QB) for h in range(HL)]
            n = len(heads)
            projg = None
            p3g = None
            for i in range(n + 3):
                sg = gen_scores(*heads[i], p2) if i < n else None
                ag = gen_av(*heads[i - 2], p2) if 2 <= i < n + 2 else None
                pre = None
                if 3 <= i < n + 3:
                    qbh = heads[i - 3]
                    pre = lambda qbh=qbh: finish_av(*qbh, p2)
                xgs = []
                if i == 0:
                    xgs.append((gen_v2(12, 13), 2))
                elif i == 1:
                    xgs.append((gen_v2(14, 15), 2))
                else:
                    # c_proj(k) spreads over the 4 slots starting one slot
                    # AFTER finish_av(k,3) lands, so the first piece's ot
                    # reads never race the reciprocal/normalize chain.
                    if i >= 7 and (i - 7) % 4 == 0:
                        projg = gen_proj((i - 7) // 4, p2)
                    if projg is not None:
                        xgs.append((projg, 1))
                if i == n:
                    p3g = gen_p3h01()
                if p3g is not None:
                    xgs.append((p3g, 1))
                if i == n + 2:
                    tailg = gen_tail()
                    xgs.append((tailg, 1))
                run_slot(sg, ag, xgs, pre)
            # drain the remaining tail pair-chains (slot 18 consumed 8 of
            # the 16 yields)
            for _ in tailg:
                pass

    nc.compile()
    nc.m = get_hw_module(nc.m)
    return nc


def kernel(hidden_states, attention_mask, w_attn, b_attn, w_proj, b_proj):
    global LAST_RESULTS
    hidden_states = np.asarray(hidden_states, dtype=np.float32)
    attention_mask = np.asarray(attention_mask, dtype=np.float32)
    w_attn = np.asarray(w_attn, dtype=np.float32)
    b_attn = np.asarray(b_attn, dtype=np.float32)
    w_proj = np.asarray(w_proj, dtype=np.float32)
    b_proj = np.asarray(b_proj, dtype=np.float32)

    use_mask = bool((attention_mask != 1.0).any())
    key = ("prog", use_mask)
    if key not in _PROG_CACHE:
        _PROG_CACHE[key] = _build_program(use_mask)
    nc = _PROG_CACHE[key]

    in_maps = []
    for c in range(N_CORES):
        b, g = divmod(c, 4)
        # xt: [D, S] -> [128, (tb, kt, tok512)]
        xt = hidden_states[b].T.astype(NPF16)                  # [D, S]
        xt_p = np.ascontiguousarray(
            xt.reshape(KT, 128, QB, 512).transpose(1, 2, 0, 3)
        ).reshape(128, QB * KT * 512)
        # wqk: [D, 1024] -> [128, (kt, f, c128)]  (kt-major: the tb0 QK
        # rounds consume one kt piece across all 8 f-chunks per round)
        wq = w_attn[:, g * FL:(g + 1) * FL]
        wk = w_attn[:, D + g * FL:D + (g + 1) * FL]
        wqk = np.concatenate([wq, wk], axis=1).astype(NPF16)   # [D, 1024]
        wqk_p = np.ascontiguousarray(
            wqk.reshape(KT, 128, 8, 128).transpose(1, 0, 2, 3)
        ).reshape(128, 8 * FW)
        # wv: [D, FL] -> [128, (kt, FL)]
        wv = w_attn[:, 2 * D + g * FL:2 * D + (g + 1) * FL].astype(NPF16)
        wv_p = np.ascontiguousarray(
            wv.reshape(KT, 128, FL).transpose(1, 0, 2)
        ).reshape(128, KT * FL)
        wp = np.ascontiguousarray(w_proj[g * FL:(g + 1) * FL, :]).astype(NPF16)
        bq = b_attn[g * FL:(g + 1) * FL]
        bk = b_attn[D + g * FL:D + (g + 1) * FL]
        bqk = np.ascontiguousarray(
            np.concatenate([bq, bk]).reshape(8, 128).T).astype(np.float32)
        kb = np.ascontiguousarray(
            attention_mask[b].reshape(KT, 128).T).astype(np.float32)
        in_maps.append({
            "xt": np.ascontiguousarray(xt_p),
            "wqk": np.ascontiguousarray(wqk_p),
            "wv": np.ascontiguousarray(wv_p),
            "wp": wp,
            "bqk": bqk,
            "kb": kb,
        })

    if not os.environ.get("KERNEL_ALLOW_TRACE"):
        os.environ["BASS_NEVER_TRACE"] = "1"
    try:
        res = bass_utils.run_bass_kernel_spmd(nc, in_maps,
                                              list(range(N_CORES)))
    except Exception:
        # Transient NRT failures can leave the axon device wedged; reset it
        # once and retry.
        try:
            import ctypes

            import jax

            jax.devices()
            _lib = ctypes.CDLL("/opt/axon/libaxon_pjrt.so")
            _lib.axon_reset.restype = ctypes.c_int64
            _lib.axon_reset()
        except Exception:
            pass
        res = bass_utils.run_bass_kernel_spmd(nc, in_maps,
                                              list(range(N_CORES)))
    LAST_RESULTS = res

    # host reduce: sum the 4 head-group partials per batch, add biases.
    # V-bias contribution: rows of A sum to 1, so each core's O gains b_v
    # per row; through c_proj that's a constant row b_v @ w_proj_slice.
    out = np.zeros((B, S, D), dtype=np.float32)
    for c in range(N_CORES):
        b, g = divmod(c, 4)
        out[b] += res.results[c]["out"].astype(np.float32)
    bias_row = b_proj.astype(np.float64).copy()
    for g in range(4):
        bv = b_attn[2 * D + g * FL:2 * D + (g + 1) * FL].astype(np.float64)
        bias_row += bv @ w_proj[g * FL:(g + 1) * FL, :].astype(np.float64)
    out += bias_row.astype(np.float32)[None, None, :]
    return out



# revision 40
# speedup vs baseline: 1.0020x; 1.0020x over previous
"""Decoder-only attention block (QKV proj + MHA + out proj) on 8 TRN2 cores.

Sharding: core c -> (batch b = c//4, head-group g = c%4). Tensor-parallel over
heads (4 of 16 heads per core), data-parallel over batch (2). Each core
computes a partial c_proj over its 512 input features; host reduces the 4
partials per batch and adds biases.

Schedule notes:
- Phase 1 is token-block-major: QK chains for block tb only need block tb's
  x columns, so the PE starts once ~2.6MB of input has landed instead of
  ~9MB. The V projection is interleaved per block; input DMA issue is spread
  across the sync/gpsimd/vector queues (DMA issue costs ~0.65us each and is
  serial per queue).
- Phase 2 runs a skew-1 software pipeline: the PE stream interleaves
  scores(head i+1) pairs with A@V(head i) pairs at matmul granularity, so
  the scalar-engine exp latency and the gpsimd softmax-denominator reduce
  never stall the PE. Any PE idle gap > ~3.4us would HAM-throttle the array
  to half clock; the pipeline is built to avoid them.
- c_proj for block k is emitted right after its 4 heads finish (PSUM tags
  are shared across phases: qk-acc/proj, v/psot); partials ship as f16.

Self-contained: hardcodes B=2, S=2048, D=2048, H=16.
"""

import os

import numpy as np

NPF16 = np.float16

import concourse.bass as bass
import concourse.bacc as bacc
import concourse.tile as tile
from concourse import mybir
from concourse import library_config
import concourse.bass_utils as bass_utils
import concourse.bass_isa as bass_isa
from concourse.bass_interp import get_hw_module

B, S, D = 2, 2048, 2048
H, DH = 16, 128
N_CORES = 8
HL = H // 4            # 4 heads per core
FL = HL * DH           # 512 local features per core
KT = D // 128          # 16 contraction tiles
TT = S // 128          # 16 token tiles
QB = S // 512          # 4 token blocks
SCALE = 1.0 / float(np.sqrt(DH))
FW = KT * 128          # 2048 cols per f-chunk of packed wqk

F16 = mybir.dt.float16
F32 = mybir.dt.float32

# Stash of the last BassKernelResults (for the local test harness only).
LAST_RESULTS = None
_PROG_CACHE = {}


def _build_program(use_mask):

    nc = bacc.Bacc("TRN2", target_bir_lowering=False, debug=False,
                   num_devices=N_CORES)

    # Host-packed layouts (see kernel() for the packing):
    #   xt  [128, (tb, kt, 512)]  x^T, token-block-major
    #   wqk [128, (f, kt, 128)]   q|k weight, f-chunk-major
    #   wv  [128, (kt, 512)]
    #   wp  [FL, D]
    xt_d = nc.dram_tensor("xt", [128, QB * KT * 512], F16, kind="ExternalInput")
    wqk_d = nc.dram_tensor("wqk", [128, 8 * FW], F16, kind="ExternalInput")
    wv_d = nc.dram_tensor("wv", [128, KT * FL], F16, kind="ExternalInput")
    wp_d = nc.dram_tensor("wp", [FL, D], F16, kind="ExternalInput")
    bqk_d = nc.dram_tensor("bqk", [128, 8], F32, kind="ExternalInput")
    kb_d = nc.dram_tensor("kb", [128, KT], F32, kind="ExternalInput")
    out_d = nc.dram_tensor("out", [S, D], F16, kind="ExternalOutput")

    xt_ap, wqk_ap, wv_ap, wp_ap = xt_d.ap(), wqk_d.ap(), wv_d.ap(), wp_d.ap()
    bqk_ap, kb_ap, out_ap = bqk_d.ap(), kb_d.ap(), out_d.ap()

    with (
        tile.TileContext(nc) as tc,
        tc.tile_pool(name="pers", bufs=1) as pers,
        tc.tile_pool(name="pwv", bufs=1) as pwv,
        tc.tile_pool(name="pxB", bufs=1) as pxB,
        tc.tile_pool(name="ps", bufs=1, space="PSUM") as ps,
    ):
        # ---- persistent tiles ----
        qt = [pers.tile([128, S], F16, tag=f"qt{h}", name=f"qt{h}") for h in range(HL)]
        ktt = [pers.tile([128, S], F16, tag=f"kt{h}", name=f"ktt{h}") for h in range(HL)]
        ot = [pers.tile([128, S], F16, tag=f"ot{h}", name=f"ot{h}") for h in range(HL)]
        vaug = [[pers.tile([128, DH], F16, tag=f"v{t}_{h}", name=f"v{t}_{h}")
                 for h in range(HL)] for t in range(TT)]
        wp_sb = [pers.tile([128, D], F16, tag=f"wp{h}", name=f"wp{h}")
                 for h in range(HL)]
        bqk_sb = pers.tile([128, 8], F32, tag="bqk", name="bqk_sb")
        kb_sb = pers.tile([128, KT], F32, tag="kb", name="kb_sb")

        nc.scalar.dma_start(bqk_sb[:], bqk_ap[:])
        if use_mask:
            nc.scalar.dma_start(kb_sb[:], kb_ap[:])

        wv_sb = pwv.tile([128, KT * FL], F16, tag="wv", name="wv_sb")
        xt3 = pxB.tile([128, KT * 512], F16, tag="xt3", name="xt3")

        state = {}

        def gen_scores(qb, h, pool):
            """scores^T -> exp -> E tiles, denominator tree + gpsimd + rcp.

            Yields 8 times (one per k-tile pair) for PE-stream interleaving.
            """
            e_tiles = []
            l1 = {}
            l2 = {}
            for p in range(KT // 2):
                pss = ps.tile([128, 1024], F32, tag="pss", bufs=2, name="pss")
                for half in range(2):
                    kt = 2 * p + half
                    nc.tensor.matmul(
                        pss[:, half * 512:(half + 1) * 512],
                        ktt[h][:, kt * 128:(kt + 1) * 128],
                        qt[h][:, qb * 512:(qb + 1) * 512],
                        start=True, stop=True,
                    )
                e = pool.tile([128, 1024], F16, tag=f"e{p}", bufs=3,
                              name=f"e{p}")
                nc.scalar.activation(
                    e[:], pss[:], mybir.ActivationFunctionType.Exp,
                    scale=SCALE,
                )
                if use_mask:
                    for half in range(2):
                        kt = 2 * p + half
                        sl = e[:, half * 512:(half + 1) * 512]
                        nc.vector.tensor_scalar_mul(sl, sl, kb_sb[:, kt:kt + 1])
                e_tiles.append(e)
                # denominator tree, spread so no round emits more than one
                # add: the c_proj stage CASTs share the vector queue and
                # gate PSUM-bank reuse ~2 pieces later, while the tree has
                # 1-2 slots of slack before the gpsimd reduce needs it.
                # The tree TAIL (l2b/dnb/dn + reduce) is emitted by gen_av
                # two slots later, one op per round, so no slot ends with
                # an add burst in front of the next slot's CASTs.
                if p % 2 == 1:
                    i = p // 2
                    t_ = pool.tile([128, 1024], F16, tag=f"l1{i % 2}", bufs=3,
                                   name=f"l1{i % 2}")
                    nc.vector.tensor_add(t_[:], e_tiles[p - 1][:],
                                         e_tiles[p][:])
                    l1[i] = t_
                if p == 4:
                    t_ = pool.tile([128, 1024], F16, tag="l2a", bufs=2,
                                   name="l2a")
                    nc.vector.tensor_add(t_[:], l1[0][:], l1[1][:])
                    l2[0] = t_
                if p == 6:
                    t_ = pool.tile([128, 512], F16, tag="dna", bufs=2,
                                   name="dna")
                    nc.vector.tensor_add(t_[:], l2[0][:, 0:512],
                                         l2[0][:, 512:1024])
                    l2["dna"] = t_
                yield
            state[(qb, h)] = (e_tiles, l1, l2["dna"])

        def gen_av(qb, h, pool):
            """O^T = V^T A^T, evicted UNNORMALIZED to a scratch tile so the
            PSUM bank never waits on the gpsimd denominator chain. The
            normalization multiply happens in finish_av once the reciprocal
            is ready (1 slot of slack). Also finishes this head's
            denominator tree (one op per round) + the gpsimd reduce.
            Yields 8 times."""
            e_tiles, l1, dna = state[(qb, h)]
            dnr = None
            psot = ps.tile([128, 512], F32, tag="pv", bufs=2, name="psot")
            for p in range(KT // 2):
                for half in range(2):
                    kt = 2 * p + half
                    nc.tensor.matmul(
                        psot[:],
                        vaug[kt][h][:],
                        e_tiles[p][:, half * 512:(half + 1) * 512],
                        start=(kt == 0), stop=(kt == KT - 1),
                    )
                if p == 0:
                    l2b = pool.tile([128, 1024], F16, tag="l2b", bufs=2,
                                    name="l2b")
                    nc.vector.tensor_add(l2b[:], l1[2][:], l1[3][:])
                elif p == 1:
                    dnb = pool.tile([128, 512], F16, tag="dnb", bufs=2,
                                    name="dnb")
                    nc.vector.tensor_add(dnb[:], l2b[:, 0:512],
                                         l2b[:, 512:1024])
                elif p == 2:
                    dn = pool.tile([128, 512], F16, tag="dn", bufs=2,
                                   name="dn")
                    nc.vector.tensor_add(dn[:], dna[:], dnb[:])
                elif p == 3:
                    dnr = pool.tile([128, 512], F32, tag="dnr", bufs=2,
                                    name="dnr")
                    nc.gpsimd.partition_all_reduce(
                        dnr[:], dn[:], channels=128,
                        reduce_op=bass_isa.ReduceOp.add)
                yield
            os_ = pool.tile([128, 512], F16, tag="osc", bufs=2, name="osc")
            nc.scalar.copy(os_[:], psot[:])
            state[(qb, h)] = (dnr, os_)

        def finish_av(qb, h, pool):
            dnr, os_ = state.pop((qb, h))
            rcp = pool.tile([128, 512], F32, tag="rcp", bufs=1, name="rcp")
            nc.vector.reciprocal_approx_fast(rcp[:], dnr[:])
            nc.vector.tensor_mul(ot[h][:, qb * 512:(qb + 1) * 512],
                                 os_[:], rcp[:])

        def gen_proj(k, pool, tail=False):
            """c_proj for token block k. Yields 32 times (2 matmuls each).

            tail=True ships per-chain [128,512] f16 DMAs alternating the
            sync/gpsimd issue queues so the final flush spreads across more
            DMA engines."""
            for ti, t in enumerate(range(k * 4, k * 4 + 4)):
                for nbp in range(2):
                    st = (None if tail else
                          pool.tile([128, 1024], F16, tag="stage", bufs=2,
                                    name="stage"))
                    for half in range(2):
                        nb = 2 * nbp + half
                        psp = ps.tile([128, 512], F32, tag="qk", bufs=2,
                                      name="psp")
                        for h in range(HL):
                            nc.tensor.matmul(
                                psp[:],
                                ot[h][:, t * 128:(t + 1) * 128],
                                wp_sb[h][:, nb * 512:(nb + 1) * 512],
                                start=(h == 0), stop=(h == HL - 1),
                                skip_group_check=True,
                            )
                            if h == 1:
                                yield
                        if tail:
                            s5 = pool.tile([128, 512], F16, tag="stg5",
                                           bufs=2, name="stg5")
                            nc.vector.tensor_copy(s5[:], psp[:])
                            eng = nc.sync if (nb + ti) % 2 == 0 else nc.gpsimd
                            eng.dma_start(
                                out_ap[t * 128:(t + 1) * 128,
                                       nb * 512:(nb + 1) * 512], s5[:])
                        else:
                            nc.vector.tensor_copy(
                                st[:, half * 512:(half + 1) * 512], psp[:])
                            if half == 1:
                                nc.sync.dma_start(
                                    out_ap[t * 128:(t + 1) * 128,
                                           nbp * 1024:(nbp + 1) * 1024],
                                    st[:])
                        yield

        def run_slot(sg, ag, xgs=(), pre=None):
            """One pipeline slot: interleave scores-pairs (sg), A@V-pairs
            (ag) and extra PE work (xgs: (generator, count) pairs — V
            chains, c_proj pieces, tail-proj partials) at matmul
            granularity. `pre` emits the reciprocal+normalize of the head
            finished 3 slots ago BEFORE any c_proj piece of this slot can
            read its ot range."""
            if pre is not None:
                pre()
            for _ in range(KT // 2):
                # xgs (c_proj pieces) go FIRST: their PSUM-evicting CASTs
                # then precede the round's denominator-tree adds in the
                # vector queue. The tree has ~2 slots of slack; the CASTs
                # gate PSUM-bank reuse two pieces later.
                for g, cnt in xgs:
                    for _ in range(cnt):
                        next(g, None)
                if sg is not None:
                    next(sg, None)
                if ag is not None:
                    next(ag, None)
            if ag is not None:
                for _ in ag:
                    pass
            if sg is not None:
                for _ in sg:
                    pass
            # xgs are NOT drained: a c_proj generator spans 4 slots (8 of
            # its 32 pieces per slot); exhaust a generator here only when
            # the caller knows it ends this slot (V chains).

        def emit_qk_chain(f, src, interleave=None):
            acc = ps.tile([128, 512], F32, tag="qk", bufs=2, name="qkacc")
            for kt in range(KT):
                nc.tensor.matmul(
                    acc[:],
                    wqk_sb[:, kt * 1024 + f * 128:kt * 1024 + (f + 1) * 128],
                    src[:, kt * 512:(kt + 1) * 512],
                    start=(kt == 0), stop=(kt == KT - 1),
                    skip_group_check=True,
                )
                if interleave is not None:
                    interleave(kt)
            dest = qt[f] if f < HL else ktt[f - HL]
            return acc, dest

        def emit_v_chain(t, src):
            psv = ps.tile([128, FL], F32, tag="pv", bufs=2, name="psv")
            for kt in range(KT):
                nc.tensor.matmul(
                    psv[:],
                    src[:, kt * 512 + (t % 4) * 128:
                        kt * 512 + (t % 4 + 1) * 128],
                    wv_sb[:, kt * FL:(kt + 1) * FL],
                    start=(kt == 0), stop=(kt == KT - 1),
                )
            for h in range(HL):
                nc.vector.tensor_copy(vaug[t][h][:],
                                      psv[:, h * 128:(h + 1) * 128])

        def gen_v2(t0, t1):
            """Two V chains for the pipeline ramp slots; 16 yields of 2
            matmuls (the trailing vaug copies are emitted before the final
            yield of each chain so 16 next() calls consume everything)."""
            for t in (t0, t1):
                psv = ps.tile([128, FL], F32, tag="pv", bufs=2, name="psv")
                for p in range(KT // 2):
                    for half in range(2):
                        kt = 2 * p + half
                        nc.tensor.matmul(
                            psv[:],
                            xt3[:, kt * 512 + (t % 4) * 128:
                                kt * 512 + (t % 4 + 1) * 128],
                            wv_sb[:, kt * FL:(kt + 1) * FL],
                            start=(kt == 0), stop=(kt == KT - 1),
                        )
                    if p == KT // 2 - 1:
                        for h in range(HL):
                            nc.vector.tensor_copy(vaug[t][h][:],
                                                  psv[:, h * 128:(h + 1) * 128])
                    yield

        # ================= phase 1: QKV projection, token-block-major =====
        with tc.tile_pool(name="pwqk", bufs=1) as pwqk:
            wqk_sb = pwqk.tile([128, 8 * FW], F16, tag="wqk", name="wqk_sb")

            with tc.tile_pool(name="pxA", bufs=1) as pxA:
                xts = [pxA.tile([128, KT * 512], F16, tag="xt0", name="xt0"),
                       pxA.tile([128, KT * 512], F16, tag="xt1", name="xt1")]

                # --- input DMA issues, spread across engine queues ---
                # wqk kt-major pieces matching the interleaved tb0 round
                # order (round kt consumes piece kt). Piece kt0 gates the
                # very first matmul: issue it FIRST on sync, in halves, so
                # the PE starts ~2us sooner; gpsimd carries kt1..15.
                nc.sync.dma_start(wqk_sb[:, 0:256], wqk_ap[:, 0:256])
                nc.sync.dma_start(xts[0][:, 0:512], xt_ap[:, 0:512])
                nc.sync.dma_start(wqk_sb[:, 256:640], wqk_ap[:, 256:640])
                nc.sync.dma_start(wqk_sb[:, 640:1024], wqk_ap[:, 640:1024])
                for kt in range(1, KT):
                    c0 = kt * 1024
                    nc.gpsimd.dma_start(wqk_sb[:, c0:c0 + 1024],
                                        wqk_ap[:, c0:c0 + 1024])
                # Preload the gpsimd ucode lib that partition_all_reduce
                # needs. Lazily it would load at phase-2 start (~7us on the
                # gpsimd queue), delaying the first softmax denominators and
                # stalling the PE. Here it fills gpsimd's idle window after
                # its DMA issues.
                nc.gpsimd.load_library(library_config.attn)
                # xt block 0 (granular, paces the first chains): split the
                # issue cost across the sync and scalar queues so all 16
                # slices land ~5us earlier; block 1 on sync
                for kt in range(1, KT // 2):
                    nc.sync.dma_start(
                        xts[0][:, kt * 512:(kt + 1) * 512],
                        xt_ap[:, kt * 512:(kt + 1) * 512])
                for kt in range(KT // 2, KT):
                    nc.scalar.dma_start(
                        xts[0][:, kt * 512:(kt + 1) * 512],
                        xt_ap[:, kt * 512:(kt + 1) * 512])
                for q in range(8):
                    b1 = KT * 512
                    nc.sync.dma_start(
                        xts[1][:, q * 1024:(q + 1) * 1024],
                        xt_ap[:, b1 + q * 1024:b1 + (q + 1) * 1024])
                # scalar (Activation) queue: wv. (wp is needed only in
                # phase 2 — issued at tb==1 to keep it out of the
                # BW-saturated ramp window.)
                for q in range(8):
                    nc.scalar.dma_start(wv_sb[:, q * 1024:(q + 1) * 1024],
                                        wv_ap[:, q * 1024:(q + 1) * 1024])

                # --- token block 0: all 8 QK chains interleaved at kt
                # granularity. Each early DMA byte feeds 8 chains, so the
                # PE reads ~225GB/s sustained instead of bursting 765GB/s
                # per chain against the ~350GB/s HBM ramp. Needs 8 open
                # PSUM groups: borrow every phase-2 tag (idle here).
                pA = ps.tile([128, 1024], F32, tag="pss", bufs=2, name="pA")
                pB = ps.tile([128, 1024], F32, tag="pss", bufs=2, name="pB")
                pC = ps.tile([128, 512], F32, tag="pv", bufs=2, name="pC")
                pD = ps.tile([128, 512], F32, tag="pv", bufs=2, name="pD")
                pE = ps.tile([128, 512], F32, tag="qk", bufs=2, name="pE")
                pF = ps.tile([128, 512], F32, tag="qk", bufs=2, name="pF")
                accs = [pA[:, 0:512], pA[:, 512:1024],
                        pB[:, 0:512], pB[:, 512:1024],
                        pC[:], pD[:], pE[:], pF[:]]
                for kt in range(KT):
                    for f in range(8):
                        nc.tensor.matmul(
                            accs[f],
                            wqk_sb[:, kt * 1024 + f * 128:
                                   kt * 1024 + (f + 1) * 128],
                            xts[0][:, kt * 512:(kt + 1) * 512],
                            start=(kt == 0), stop=(kt == KT - 1),
                            skip_group_check=True,
                        )
                # evict f=6,7 first: their qk-tag banks gate tb1's first
                # two chains.
                for f in (6, 7, 0, 1, 2, 3, 4, 5):
                    dest = qt[f] if f < HL else ktt[f - HL]
                    nc.vector.tensor_scalar_add(dest[:, 0:512], accs[f],
                                                bqk_sb[:, f:f + 1])
                for t in range(4):
                    emit_v_chain(t, xts[0])
                # prefetch block 2 into the rotated xt0 buffer
                xts.append(pxA.tile([128, KT * 512], F16, tag="xt0",
                                    name="xt0"))
                b2 = 2 * KT * 512
                for q in range(8):
                    nc.sync.dma_start(
                        xts[2][:, q * 1024:(q + 1) * 1024],
                        xt_ap[:, b2 + q * 1024:b2 + (q + 1) * 1024])

                # --- token blocks 1..2, chain-major (data is ahead now) ---
                for tb in range(1, 3):
                    src = xts[tb]
                    for f in range(8):
                        acc, dest = emit_qk_chain(f, src)
                        nc.vector.tensor_scalar_add(
                            dest[:, tb * 512:(tb + 1) * 512],
                            acc[:], bqk_sb[:, f:f + 1])
                    for t in range(4 * tb, 4 * tb + 4):
                        emit_v_chain(t, src)
                    if tb == 1:
                        b3 = 3 * KT * 512
                        for q in range(8):
                            nc.sync.dma_start(
                                xt3[:, q * 1024:(q + 1) * 1024],
                                xt_ap[:, b3 + q * 1024:b3 + (q + 1) * 1024])
                        for h in range(HL):
                            nc.scalar.dma_start(
                                wp_sb[h][:], wp_ap[h * 128:(h + 1) * 128, :])
            # pxA closed (blocks 0-2 consumed)

            # --- block 3: QK only; its V chains move into the phase-2
            # ramp slots so the pipeline starts with full 3-lane units ---
            for f in range(8):
                acc, dest = emit_qk_chain(f, xt3)
                nc.vector.tensor_scalar_add(dest[:, 3 * 512:4 * 512], acc[:],
                                            bqk_sb[:, f:f + 1])
        # pwqk closed

        def gen_p3h01():
            """c_proj(3) partial chains over heads 0-1, staged into the
            dead qt tiles (all scores done by slot 16). 8 pair-units (two
            nb half-chains into one [128,1024] PSUM tile, one CAST), 2
            matmuls per yield, filling the PE time freed by the finished
            scores lane in slots 16-17. The tail then only runs the h=2,3
            half-chains plus an add."""
            for u in range(8):
                ti, pr = divmod(u, 2)
                t = 12 + ti
                pp = ps.tile([128, 1024], F32, tag="pss", bufs=2,
                             name="pp3")
                for half in range(2):
                    nb = 2 * pr + half
                    for hh in (0, 1):
                        nc.tensor.matmul(
                            pp[:, half * 512:(half + 1) * 512],
                            ot[hh][:, t * 128:(t + 1) * 128],
                            wp_sb[hh][:, nb * 512:(nb + 1) * 512],
                            start=(hh == 0), stop=(hh == 1),
                            skip_group_check=True,
                        )
                    if half == 0:
                        yield
                nc.vector.tensor_copy(qt[ti][:, pr * 1024:(pr + 1) * 1024],
                                      pp[:])
                yield

        def gen_tail():
            """c_proj(3) h=2,3 half-chains added onto the staged h=0,1
            partials, shipped per [128,1024] chunk the moment its add
            lands (issue queues alternate so transfers overlap). Runs
            half inside slot 18, half drained after — the adds (1.2us
            each on vector) would oversubscribe a single slot. 2 matmuls
            per yield, 16 yields."""
            for u in range(8):
                ti, pair = divmod(u, 2)
                t = 12 + ti
                s5 = p2.tile([128, 1024], F16, tag="stg5", bufs=3,
                             name="stg5")
                pp = ps.tile([128, 1024], F32, tag="pss", bufs=2,
                             name="pp5")
                for half in range(2):
                    nb = 2 * pair + half
                    for hh in (2, 3):
                        nc.tensor.matmul(
                            pp[:, half * 512:(half + 1) * 512],
                            ot[hh][:, t * 128:(t + 1) * 128],
                            wp_sb[hh][:, nb * 512:(nb + 1) * 512],
                            start=(hh == 2), stop=(hh == 3),
                            skip_group_check=True,
                        )
                    if half == 0:
                        yield
                stg = qt[ti][:, pair * 1024:(pair + 1) * 1024]
                nc.vector.tensor_add(s5[:], stg, pp[:])
                eng = (nc.sync, nc.scalar)[u % 2]
                eng.dma_start(
                    out_ap[t * 128:(t + 1) * 128,
                           pair * 1024:(pair + 1) * 1024], s5[:])
                yield

        # ================= phase 2: skew-2 three-lane pipeline ============
        # slot i: scores(H_i) | A@V(H_{i-2}) | extra (V chains in the two
        # ramp slots, c_proj(k) spread over the four slots after av(k,3)).
        with tc.tile_pool(name="p2", bufs=1) as p2:
            heads = [(k, h) for k in range(QB) for h in range(HL)]
            n = len(heads)
            projg = None
            p3g = None
            for i in range(n + 3):
                sg = gen_scores(*heads[i], p2) if i < n else None
                ag = gen_av(*heads[i - 2], p2) if 2 <= i < n + 2 else None
                pre = None
                if 3 <= i < n + 3:
                    qbh = heads[i - 3]
                    pre = lambda qbh=qbh: finish_av(*qbh, p2)
                xgs = []
                if i == 0:
                    xgs.append((gen_v2(12, 13), 2))
                elif i == 1:
                    xgs.append((gen_v2(14, 15), 2))
                else:
                    # c_proj(k) spreads over the 4 slots starting one slot
                    # AFTER finish_av(k,3) lands, so the first piece's ot
                    # reads never race the reciprocal/normalize chain.
                    if i >= 7 and (i - 7) % 4 == 0:
                        projg = gen_proj((i - 7) // 4, p2)
                    if projg is not None:
                        xgs.append((projg, 1))
                if i == n:
                    p3g = gen_p3h01()
                if p3g is not None:
                    xgs.append((p3g, 1))
                if i == n + 2:
                    tailg = gen_tail()
                    xgs.append((tailg, 1))
                run_slot(sg, ag, xgs, pre)
            # drain the remaining tail pair-chains (slot 18 consumed 8 of
            # the 16 yields)
            for _ in tailg:
                pass

    nc.compile()
    nc.m = get_hw_module(nc.m)
    return nc


def kernel(hidden_states, attention_mask, w_attn, b_attn, w_proj, b_proj):
    global LAST_RESULTS
    hidden_states = np.asarray(hidden_states, dtype=np.float32)
    attention_mask = np.asarray(attention_mask, dtype=np.float32)
    w_attn = np.asarray(w_attn, dtype=np.float32)
    b_attn = np.asarray(b_attn, dtype=np.float32)
    w_proj = np.asarray(w_proj, dtype=np.float32)
    b_proj = np.asarray(b_proj, dtype=np.float32)

    use_mask = bool((attention_mask != 1.0).any())
    key = ("prog", use_mask)
    if key not in _PROG_CACHE:
        _PROG_CACHE[key] = _build_program(use_mask)
    nc = _PROG_CACHE[key]

    in_maps = []
    for c in range(N_CORES):
        b, g = divmod(c, 4)
        # xt: [D, S] -> [128, (tb, kt, tok512)]
        xt = hidden_states[b].T.astype(NPF16)                  # [D, S]
        xt_p = np.ascontiguousarray(
            xt.reshape(KT, 128, QB, 512).transpose(1, 2, 0, 3)
        ).reshape(128, QB * KT * 512)
        # wqk: [D, 1024] -> [128, (kt, f, c128)]  (kt-major: the tb0 QK
        # rounds consume one kt piece across all 8 f-chunks per round)
        wq = w_attn[:, g * FL:(g + 1) * FL]
        wk = w_attn[:, D + g * FL:D + (g + 1) * FL]
        wqk = np.concatenate([wq, wk], axis=1).astype(NPF16)   # [D, 1024]
        wqk_p = np.ascontiguousarray(
            wqk.reshape(KT, 128, 8, 128).transpose(1, 0, 2, 3)
        ).reshape(128, 8 * FW)
        # wv: [D, FL] -> [128, (kt, FL)]
        wv = w_attn[:, 2 * D + g * FL:2 * D + (g + 1) * FL].astype(NPF16)
        wv_p = np.ascontiguousarray(
            wv.reshape(KT, 128, FL).transpose(1, 0, 2)
        ).reshape(128, KT * FL)
        wp = np.ascontiguousarray(w_proj[g * FL:(g + 1) * FL, :]).astype(NPF16)
        bq = b_attn[g * FL:(g + 1) * FL]
        bk = b_attn[D + g * FL:D + (g + 1) * FL]
        bqk = np.ascontiguousarray(
            np.concatenate([bq, bk]).reshape(8, 128).T).astype(np.float32)
        kb = np.ascontiguousarray(
            attention_mask[b].reshape(KT, 128).T).astype(np.float32)
        in_maps.append({
            "xt": np.ascontiguousarray(xt_p),
            "wqk": np.ascontiguousarray(wqk_p),
            "wv": np.ascontiguousarray(wv_p),
            "wp": wp,
            "bqk": bqk,
            "kb": kb,
        })

    if not os.environ.get("KERNEL_ALLOW_TRACE"):
        os.environ["BASS_NEVER_TRACE"] = "1"
    try:
        res = bass_utils.run_bass_kernel_spmd(nc, in_maps,
                                              list(range(N_CORES)))
    except Exception:
        # Transient NRT failures can leave the axon device wedged; reset it
        # once and retry.
        try:
            import ctypes

            import jax

            jax.devices()
            _lib = ctypes.CDLL("/opt/axon/libaxon_pjrt.so")
            _lib.axon_reset.restype = ctypes.c_int64
            _lib.axon_reset()
        except Exception:
            pass
        res = bass_utils.run_bass_kernel_spmd(nc, in_maps,
                                              list(range(N_CORES)))
    LAST_RESULTS = res

    # host reduce: sum the 4 head-group partials per batch, add biases.
    # V-bias contribution: rows of A sum to 1, so each core's O gains b_v
    # per row; through c_proj that's a constant row b_v @ w_proj_slice.
    out = np.zeros((B, S, D), dtype=np.float32)
    for c in range(N_CORES):
        b, g = divmod(c, 4)
        out[b] += res.results[c]["out"].astype(np.float32)
    bias_row = b_proj.astype(np.float64).copy()
    for g in range(4):
        bv = b_attn[2 * D + g * FL:2 * D + (g + 1) * FL].astype(np.float64)
        bias_row += bv @ w_proj[g * FL:(g + 1) * FL, :].astype(np.float64)
    out += bias_row.astype(np.float32)[None, None, :]
    return out



# revision 41
# speedup vs baseline: 1.0080x; 1.0059x over previous
"""Decoder-only attention block (QKV proj + MHA + out proj) on 8 TRN2 cores.

Sharding: core c -> (batch b = c//4, head-group g = c%4). Tensor-parallel over
heads (4 of 16 heads per core), data-parallel over batch (2). Each core
computes a partial c_proj over its 512 input features; host reduces the 4
partials per batch and adds biases.

Schedule notes:
- Phase 1 is token-block-major: QK chains for block tb only need block tb's
  x columns, so the PE starts once ~2.6MB of input has landed instead of
  ~9MB. The V projection is interleaved per block; input DMA issue is spread
  across the sync/gpsimd/vector queues (DMA issue costs ~0.65us each and is
  serial per queue).
- Phase 2 runs a skew-1 software pipeline: the PE stream interleaves
  scores(head i+1) pairs with A@V(head i) pairs at matmul granularity, so
  the scalar-engine exp latency and the gpsimd softmax-denominator reduce
  never stall the PE. Any PE idle gap > ~3.4us would HAM-throttle the array
  to half clock; the pipeline is built to avoid them.
- c_proj for block k is emitted right after its 4 heads finish (PSUM tags
  are shared across phases: qk-acc/proj, v/psot); partials ship as f16.

Self-contained: hardcodes B=2, S=2048, D=2048, H=16.
"""

import os

import numpy as np

NPF16 = np.float16

import concourse.bass as bass
import concourse.bacc as bacc
import concourse.tile as tile
from concourse import mybir
from concourse import library_config
import concourse.bass_utils as bass_utils
import concourse.bass_isa as bass_isa
from concourse.bass_interp import get_hw_module

B, S, D = 2, 2048, 2048
H, DH = 16, 128
N_CORES = 8
HL = H // 4            # 4 heads per core
FL = HL * DH           # 512 local features per core
KT = D // 128          # 16 contraction tiles
TT = S // 128          # 16 token tiles
QB = S // 512          # 4 token blocks
SCALE = 1.0 / float(np.sqrt(DH))
FW = KT * 128          # 2048 cols per f-chunk of packed wqk

F16 = mybir.dt.float16
F32 = mybir.dt.float32

# Stash of the last BassKernelResults (for the local test harness only).
LAST_RESULTS = None
_PROG_CACHE = {}


def _build_program(use_mask):

    nc = bacc.Bacc("TRN2", target_bir_lowering=False, debug=False,
                   num_devices=N_CORES)

    # Host-packed layouts (see kernel() for the packing):
    #   xt  [128, (tb, kt, 512)]  x^T, token-block-major
    #   wqk [128, (f, kt, 128)]   q|k weight, f-chunk-major
    #   wv  [128, (kt, 512)]
    #   wp  [FL, D]
    xt_d = nc.dram_tensor("xt", [128, QB * KT * 512], F16, kind="ExternalInput")
    wqk_d = nc.dram_tensor("wqk", [128, 8 * FW], F16, kind="ExternalInput")
    wv_d = nc.dram_tensor("wv", [128, KT * FL], F16, kind="ExternalInput")
    wp_d = nc.dram_tensor("wp", [FL, D], F16, kind="ExternalInput")
    bqk_d = nc.dram_tensor("bqk", [128, 8], F32, kind="ExternalInput")
    kb_d = nc.dram_tensor("kb", [128, KT], F32, kind="ExternalInput")
    out_d = nc.dram_tensor("out", [S, D], F16, kind="ExternalOutput")

    xt_ap, wqk_ap, wv_ap, wp_ap = xt_d.ap(), wqk_d.ap(), wv_d.ap(), wp_d.ap()
    bqk_ap, kb_ap, out_ap = bqk_d.ap(), kb_d.ap(), out_d.ap()

    with (
        tile.TileContext(nc) as tc,
        tc.tile_pool(name="pers", bufs=1) as pers,
        tc.tile_pool(name="pwv", bufs=1) as pwv,
        tc.tile_pool(name="pxB", bufs=1) as pxB,
        tc.tile_pool(name="ps", bufs=1, space="PSUM") as ps,
    ):
        # ---- persistent tiles ----
        qt = [pers.tile([128, S], F16, tag=f"qt{h}", name=f"qt{h}") for h in range(HL)]
        ktt = [pers.tile([128, S], F16, tag=f"kt{h}", name=f"ktt{h}") for h in range(HL)]
        ot = [pers.tile([128, S], F16, tag=f"ot{h}", name=f"ot{h}") for h in range(HL)]
        vaug = [[pers.tile([128, DH], F16, tag=f"v{t}_{h}", name=f"v{t}_{h}")
                 for h in range(HL)] for t in range(TT)]
        wp_sb = [pers.tile([128, D], F16, tag=f"wp{h}", name=f"wp{h}")
                 for h in range(HL)]
        bqk_sb = pers.tile([128, 8], F32, tag="bqk", name="bqk_sb")
        kb_sb = pers.tile([128, KT], F32, tag="kb", name="kb_sb")

        nc.scalar.dma_start(bqk_sb[:], bqk_ap[:])
        if use_mask:
            nc.scalar.dma_start(kb_sb[:], kb_ap[:])

        wv_sb = pwv.tile([128, KT * FL], F16, tag="wv", name="wv_sb")
        xt3 = pxB.tile([128, KT * 512], F16, tag="xt3", name="xt3")

        state = {}

        def gen_scores(qb, h, pool):
            """scores^T -> exp -> E tiles, denominator tree + gpsimd + rcp.

            Yields 8 times (one per k-tile pair) for PE-stream interleaving.
            """
            e_tiles = []
            l1 = {}
            l2 = {}
            for p in range(KT // 2):
                pss = ps.tile([128, 1024], F32, tag="pss", bufs=2, name="pss")
                for half in range(2):
                    kt = 2 * p + half
                    nc.tensor.matmul(
                        pss[:, half * 512:(half + 1) * 512],
                        ktt[h][:, kt * 128:(kt + 1) * 128],
                        qt[h][:, qb * 512:(qb + 1) * 512],
                        start=True, stop=True,
                    )
                e = pool.tile([128, 1024], F16, tag=f"e{p}", bufs=3,
                              name=f"e{p}")
                nc.scalar.activation(
                    e[:], pss[:], mybir.ActivationFunctionType.Exp,
                    scale=SCALE,
                )
                if use_mask:
                    for half in range(2):
                        kt = 2 * p + half
                        sl = e[:, half * 512:(half + 1) * 512]
                        nc.vector.tensor_scalar_mul(sl, sl, kb_sb[:, kt:kt + 1])
                e_tiles.append(e)
                # denominator tree, spread so no round emits more than one
                # add: the c_proj stage CASTs share the vector queue and
                # gate PSUM-bank reuse ~2 pieces later, while the tree has
                # 1-2 slots of slack before the gpsimd reduce needs it.
                # The tree TAIL (l2b/dnb/dn + reduce) is emitted by gen_av
                # two slots later, one op per round, so no slot ends with
                # an add burst in front of the next slot's CASTs.
                if p % 2 == 1:
                    i = p // 2
                    t_ = pool.tile([128, 1024], F16, tag=f"l1{i % 2}", bufs=3,
                                   name=f"l1{i % 2}")
                    nc.vector.tensor_add(t_[:], e_tiles[p - 1][:],
                                         e_tiles[p][:])
                    l1[i] = t_
                if p == 4:
                    t_ = pool.tile([128, 1024], F16, tag="l2a", bufs=2,
                                   name="l2a")
                    nc.vector.tensor_add(t_[:], l1[0][:], l1[1][:])
                    l2[0] = t_
                if p == 6:
                    t_ = pool.tile([128, 512], F16, tag="dna", bufs=2,
                                   name="dna")
                    nc.vector.tensor_add(t_[:], l2[0][:, 0:512],
                                         l2[0][:, 512:1024])
                    l2["dna"] = t_
                yield
            state[(qb, h)] = (e_tiles, l1, l2["dna"])

        def gen_av(qb, h, pool):
            """O^T = V^T A^T, evicted UNNORMALIZED to a scratch tile so the
            PSUM bank never waits on the gpsimd denominator chain. The
            normalization multiply happens in finish_av once the reciprocal
            is ready (1 slot of slack). Also finishes this head's
            denominator tree (one op per round) + the gpsimd reduce.
            Yields 8 times."""
            e_tiles, l1, dna = state[(qb, h)]
            dnr = None
            psot = ps.tile([128, 512], F32, tag="pv", bufs=2, name="psot")
            for p in range(KT // 2):
                for half in range(2):
                    kt = 2 * p + half
                    nc.tensor.matmul(
                        psot[:],
                        vaug[kt][h][:],
                        e_tiles[p][:, half * 512:(half + 1) * 512],
                        start=(kt == 0), stop=(kt == KT - 1),
                    )
                if p == 0:
                    l2b = pool.tile([128, 1024], F16, tag="l2b", bufs=2,
                                    name="l2b")
                    nc.vector.tensor_add(l2b[:], l1[2][:], l1[3][:])
                elif p == 1:
                    dnb = pool.tile([128, 512], F16, tag="dnb", bufs=2,
                                    name="dnb")
                    nc.vector.tensor_add(dnb[:], l2b[:, 0:512],
                                         l2b[:, 512:1024])
                elif p == 2:
                    dn = pool.tile([128, 512], F16, tag="dn", bufs=2,
                                   name="dn")
                    nc.vector.tensor_add(dn[:], dna[:], dnb[:])
                elif p == 3:
                    dnr = pool.tile([128, 512], F32, tag="dnr", bufs=2,
                                    name="dnr")
                    nc.gpsimd.partition_all_reduce(
                        dnr[:], dn[:], channels=128,
                        reduce_op=bass_isa.ReduceOp.add)
                yield
            os_ = pool.tile([128, 512], F16, tag="osc", bufs=2, name="osc")
            nc.scalar.copy(os_[:], psot[:])
            state[(qb, h)] = (dnr, os_)

        def finish_av(qb, h, pool):
            dnr, os_ = state.pop((qb, h))
            rcp = pool.tile([128, 512], F32, tag="rcp", bufs=1, name="rcp")
            nc.vector.reciprocal_approx_fast(rcp[:], dnr[:])
            nc.vector.tensor_mul(ot[h][:, qb * 512:(qb + 1) * 512],
                                 os_[:], rcp[:])

        def gen_proj(k, pool, tail=False):
            """c_proj for token block k. Yields 32 times (2 matmuls each).

            tail=True ships per-chain [128,512] f16 DMAs alternating the
            sync/gpsimd issue queues so the final flush spreads across more
            DMA engines."""
            for ti, t in enumerate(range(k * 4, k * 4 + 4)):
                for nbp in range(2):
                    st = (None if tail else
                          pool.tile([128, 1024], F16, tag="stage", bufs=2,
                                    name="stage"))
                    for half in range(2):
                        nb = 2 * nbp + half
                        psp = ps.tile([128, 512], F32, tag="qk", bufs=2,
                                      name="psp")
                        for h in range(HL):
                            nc.tensor.matmul(
                                psp[:],
                                ot[h][:, t * 128:(t + 1) * 128],
                                wp_sb[h][:, nb * 512:(nb + 1) * 512],
                                start=(h == 0), stop=(h == HL - 1),
                                skip_group_check=True,
                            )
                            if h == 1:
                                yield
                        if tail:
                            s5 = pool.tile([128, 512], F16, tag="stg5",
                                           bufs=2, name="stg5")
                            nc.vector.tensor_copy(s5[:], psp[:])
                            eng = nc.sync if (nb + ti) % 2 == 0 else nc.gpsimd
                            eng.dma_start(
                                out_ap[t * 128:(t + 1) * 128,
                                       nb * 512:(nb + 1) * 512], s5[:])
                        else:
                            nc.vector.tensor_copy(
                                st[:, half * 512:(half + 1) * 512], psp[:])
                            if half == 1:
                                nc.sync.dma_start(
                                    out_ap[t * 128:(t + 1) * 128,
                                           nbp * 1024:(nbp + 1) * 1024],
                                    st[:])
                        yield

        def run_slot(sg, ag, xgs=(), pre=None):
            """One pipeline slot: interleave scores-pairs (sg), A@V-pairs
            (ag) and extra PE work (xgs: (generator, count) pairs — V
            chains, c_proj pieces, tail-proj partials) at matmul
            granularity. `pre` emits the reciprocal+normalize of the head
            finished 3 slots ago BEFORE any c_proj piece of this slot can
            read its ot range."""
            if pre is not None:
                pre()
            for _ in range(KT // 2):
                # xgs (c_proj pieces) go FIRST: their PSUM-evicting CASTs
                # then precede the round's denominator-tree adds in the
                # vector queue. The tree has ~2 slots of slack; the CASTs
                # gate PSUM-bank reuse two pieces later.
                for g, cnt in xgs:
                    for _ in range(cnt):
                        next(g, None)
                if sg is not None:
                    next(sg, None)
                if ag is not None:
                    next(ag, None)
            if ag is not None:
                for _ in ag:
                    pass
            if sg is not None:
                for _ in sg:
                    pass
            # xgs are NOT drained: a c_proj generator spans 4 slots (8 of
            # its 32 pieces per slot); exhaust a generator here only when
            # the caller knows it ends this slot (V chains).

        def emit_qk_chain(f, src, interleave=None):
            acc = ps.tile([128, 512], F32, tag="qk", bufs=2, name="qkacc")
            for kt in range(KT):
                nc.tensor.matmul(
                    acc[:],
                    wqk_sb[:, kt * 1024 + f * 128:kt * 1024 + (f + 1) * 128],
                    src[:, kt * 512:(kt + 1) * 512],
                    start=(kt == 0), stop=(kt == KT - 1),
                    skip_group_check=True,
                )
                if interleave is not None:
                    interleave(kt)
            dest = qt[f] if f < HL else ktt[f - HL]
            return acc, dest

        def emit_v_chain(t, src):
            psv = ps.tile([128, FL], F32, tag="pv", bufs=2, name="psv")
            for kt in range(KT):
                nc.tensor.matmul(
                    psv[:],
                    src[:, kt * 512 + (t % 4) * 128:
                        kt * 512 + (t % 4 + 1) * 128],
                    wv_sb[:, kt * FL:(kt + 1) * FL],
                    start=(kt == 0), stop=(kt == KT - 1),
                )
            for h in range(HL):
                nc.vector.tensor_copy(vaug[t][h][:],
                                      psv[:, h * 128:(h + 1) * 128])

        def gen_v2(t0, t1):
            """Two V chains for the pipeline ramp slots; 16 yields of 2
            matmuls (the trailing vaug copies are emitted before the final
            yield of each chain so 16 next() calls consume everything)."""
            for t in (t0, t1):
                psv = ps.tile([128, FL], F32, tag="pv", bufs=2, name="psv")
                for p in range(KT // 2):
                    for half in range(2):
                        kt = 2 * p + half
                        nc.tensor.matmul(
                            psv[:],
                            xt3[:, kt * 512 + (t % 4) * 128:
                                kt * 512 + (t % 4 + 1) * 128],
                            wv_sb[:, kt * FL:(kt + 1) * FL],
                            start=(kt == 0), stop=(kt == KT - 1),
                        )
                    if p == KT // 2 - 1:
                        for h in range(HL):
                            nc.vector.tensor_copy(vaug[t][h][:],
                                                  psv[:, h * 128:(h + 1) * 128])
                    yield

        # ================= phase 1: QKV projection, token-block-major =====
        with tc.tile_pool(name="pwqk", bufs=1) as pwqk:
            wqk_sb = pwqk.tile([128, 8 * FW], F16, tag="wqk", name="wqk_sb")

            with tc.tile_pool(name="pxA", bufs=1) as pxA:
                xts = [pxA.tile([128, KT * 512], F16, tag="xt0", name="xt0"),
                       pxA.tile([128, KT * 512], F16, tag="xt1", name="xt1")]

                # --- input DMA issues, spread across engine queues ---
                # wqk kt-major pieces matching the interleaved tb0 round
                # order (round kt consumes piece kt). Piece kt0 gates the
                # very first matmul: issue it FIRST on sync, in halves, so
                # the PE starts ~2us sooner; gpsimd carries kt1..15.
                nc.sync.dma_start(wqk_sb[:, 0:512], wqk_ap[:, 0:512])
                nc.sync.dma_start(xts[0][:, 0:512], xt_ap[:, 0:512])
                nc.sync.dma_start(wqk_sb[:, 512:1024], wqk_ap[:, 512:1024])
                for kt in range(1, KT):
                    c0 = kt * 1024
                    nc.gpsimd.dma_start(wqk_sb[:, c0:c0 + 1024],
                                        wqk_ap[:, c0:c0 + 1024])
                # Preload the gpsimd ucode lib that partition_all_reduce
                # needs. Lazily it would load at phase-2 start (~7us on the
                # gpsimd queue), delaying the first softmax denominators and
                # stalling the PE. Here it fills gpsimd's idle window after
                # its DMA issues.
                nc.gpsimd.load_library(library_config.attn)
                # xt block 0 (granular, paces the first chains): split the
                # issue cost across the sync and scalar queues so all 16
                # slices land ~5us earlier; block 1 on sync
                for kt in range(1, KT // 2):
                    nc.sync.dma_start(
                        xts[0][:, kt * 512:(kt + 1) * 512],
                        xt_ap[:, kt * 512:(kt + 1) * 512])
                for kt in range(KT // 2, KT):
                    nc.scalar.dma_start(
                        xts[0][:, kt * 512:(kt + 1) * 512],
                        xt_ap[:, kt * 512:(kt + 1) * 512])
                for q in range(8):
                    b1 = KT * 512
                    nc.sync.dma_start(
                        xts[1][:, q * 1024:(q + 1) * 1024],
                        xt_ap[:, b1 + q * 1024:b1 + (q + 1) * 1024])
                # scalar (Activation) queue: wv. (wp is needed only in
                # phase 2 — issued at tb==1 to keep it out of the
                # BW-saturated ramp window.)
                for q in range(8):
                    nc.scalar.dma_start(wv_sb[:, q * 1024:(q + 1) * 1024],
                                        wv_ap[:, q * 1024:(q + 1) * 1024])

                # --- token block 0: all 8 QK chains interleaved at kt
                # granularity. Each early DMA byte feeds 8 chains, so the
                # PE reads ~225GB/s sustained instead of bursting 765GB/s
                # per chain against the ~350GB/s HBM ramp. Needs 8 open
                # PSUM groups: borrow every phase-2 tag (idle here).
                pA = ps.tile([128, 1024], F32, tag="pss", bufs=2, name="pA")
                pB = ps.tile([128, 1024], F32, tag="pss", bufs=2, name="pB")
                pC = ps.tile([128, 512], F32, tag="pv", bufs=2, name="pC")
                pD = ps.tile([128, 512], F32, tag="pv", bufs=2, name="pD")
                pE = ps.tile([128, 512], F32, tag="qk", bufs=2, name="pE")
                pF = ps.tile([128, 512], F32, tag="qk", bufs=2, name="pF")
                accs = [pA[:, 0:512], pA[:, 512:1024],
                        pB[:, 0:512], pB[:, 512:1024],
                        pC[:], pD[:], pE[:], pF[:]]
                for kt in range(KT):
                    for f in range(8):
                        nc.tensor.matmul(
                            accs[f],
                            wqk_sb[:, kt * 1024 + f * 128:
                                   kt * 1024 + (f + 1) * 128],
                            xts[0][:, kt * 512:(kt + 1) * 512],
                            start=(kt == 0), stop=(kt == KT - 1),
                            skip_group_check=True,
                        )
                # evict f=6,7 first: their qk-tag banks gate tb1's first
                # two chains.
                for f in (6, 7, 0, 1, 2, 3, 4, 5):
                    dest = qt[f] if f < HL else ktt[f - HL]
                    nc.vector.tensor_scalar_add(dest[:, 0:512], accs[f],
                                                bqk_sb[:, f:f + 1])
                for t in range(4):
                    emit_v_chain(t, xts[0])
                # prefetch block 2 into the rotated xt0 buffer
                xts.append(pxA.tile([128, KT * 512], F16, tag="xt0",
                                    name="xt0"))
                b2 = 2 * KT * 512
                for q in range(8):
                    nc.sync.dma_start(
                        xts[2][:, q * 1024:(q + 1) * 1024],
                        xt_ap[:, b2 + q * 1024:b2 + (q + 1) * 1024])

                # --- token blocks 1..2, chain-major (data is ahead now) ---
                for tb in range(1, 3):
                    src = xts[tb]
                    for f in range(8):
                        acc, dest = emit_qk_chain(f, src)
                        nc.vector.tensor_scalar_add(
                            dest[:, tb * 512:(tb + 1) * 512],
                            acc[:], bqk_sb[:, f:f + 1])
                    for t in range(4 * tb, 4 * tb + 4):
                        emit_v_chain(t, src)
                    if tb == 1:
                        b3 = 3 * KT * 512
                        for q in range(8):
                            nc.sync.dma_start(
                                xt3[:, q * 1024:(q + 1) * 1024],
                                xt_ap[:, b3 + q * 1024:b3 + (q + 1) * 1024])
                        for h in range(HL):
                            nc.scalar.dma_start(
                                wp_sb[h][:], wp_ap[h * 128:(h + 1) * 128, :])
            # pxA closed (blocks 0-2 consumed)

            # --- block 3: QK only; its V chains move into the phase-2
            # ramp slots so the pipeline starts with full 3-lane units ---
            for f in range(8):
                acc, dest = emit_qk_chain(f, xt3)
                nc.vector.tensor_scalar_add(dest[:, 3 * 512:4 * 512], acc[:],
                                            bqk_sb[:, f:f + 1])
        # pwqk closed

        def gen_p3h01():
            """c_proj(3) partial chains over heads 0-1, staged into the
            dead qt tiles (all scores done by slot 16). 8 pair-units (two
            nb half-chains into one [128,1024] PSUM tile, one CAST), 2
            matmuls per yield, filling the PE time freed by the finished
            scores lane in slots 16-17. The tail then only runs the h=2,3
            half-chains plus an add."""
            for u in range(8):
                ti, pr = divmod(u, 2)
                t = 12 + ti
                pp = ps.tile([128, 1024], F32, tag="pss", bufs=2,
                             name="pp3")
                for half in range(2):
                    nb = 2 * pr + half
                    for hh in (0, 1):
                        nc.tensor.matmul(
                            pp[:, half * 512:(half + 1) * 512],
                            ot[hh][:, t * 128:(t + 1) * 128],
                            wp_sb[hh][:, nb * 512:(nb + 1) * 512],
                            start=(hh == 0), stop=(hh == 1),
                            skip_group_check=True,
                        )
                    if half == 0:
                        yield
                nc.vector.tensor_copy(qt[ti][:, pr * 1024:(pr + 1) * 1024],
                                      pp[:])
                yield

        def gen_tail():
            """c_proj(3) h=2,3 half-chains added onto the staged h=0,1
            partials, shipped per [128,1024] chunk the moment its add
            lands (issue queues alternate so transfers overlap). Runs
            half inside slot 18, half drained after — the adds (1.2us
            each on vector) would oversubscribe a single slot. 2 matmuls
            per yield, 16 yields."""
            for u in range(8):
                ti, pair = divmod(u, 2)
                t = 12 + ti
                s5 = p2.tile([128, 1024], F16, tag="stg5", bufs=3,
                             name="stg5")
                pp = ps.tile([128, 1024], F32, tag="pss", bufs=2,
                             name="pp5")
                for half in range(2):
                    nb = 2 * pair + half
                    for hh in (2, 3):
                        nc.tensor.matmul(
                            pp[:, half * 512:(half + 1) * 512],
                            ot[hh][:, t * 128:(t + 1) * 128],
                            wp_sb[hh][:, nb * 512:(nb + 1) * 512],
                            start=(hh == 2), stop=(hh == 3),
                            skip_group_check=True,
                        )
                    if half == 0:
                        yield
                stg = qt[ti][:, pair * 1024:(pair + 1) * 1024]
                nc.vector.tensor_add(s5[:], stg, pp[:])
                eng = (nc.sync, nc.scalar)[u % 2]
                eng.dma_start(
                    out_ap[t * 128:(t + 1) * 128,
                           pair * 1024:(pair + 1) * 1024], s5[:])
                yield

        # ================= phase 2: skew-2 three-lane pipeline ============
        # slot i: scores(H_i) | A@V(H_{i-2}) | extra (V chains in the two
        # ramp slots, c_proj(k) spread over the four slots after av(k,3)).
        with tc.tile_pool(name="p2", bufs=1) as p2:
            heads = [(k, h) for k in range(QB) for h in range(HL)]
            n = len(heads)
            projg = None
            p3g = None
            for i in range(n + 3):
                sg = gen_scores(*heads[i], p2) if i < n else None
                ag = gen_av(*heads[i - 2], p2) if 2 <= i < n + 2 else None
                pre = None
                if 3 <= i < n + 3:
                    qbh = heads[i - 3]
                    pre = lambda qbh=qbh: finish_av(*qbh, p2)
                xgs = []
                if i == 0:
                    xgs.append((gen_v2(12, 13), 2))
                elif i == 1:
                    xgs.append((gen_v2(14, 15), 2))
                else:
                    # c_proj(k) spreads over the 4 slots starting one slot
                    # AFTER finish_av(k,3) lands, so the first piece's ot
                    # reads never race the reciprocal/normalize chain.
                    if i >= 7 and (i - 7) % 4 == 0:
                        projg = gen_proj((i - 7) // 4, p2)
                    if projg is not None:
                        xgs.append((projg, 1))
                if i == n:
                    p3g = gen_p3h01()
                if p3g is not None:
                    xgs.append((p3g, 1))
                if i == n + 2:
                    tailg = gen_tail()
                    xgs.append((tailg, 1))
                run_slot(sg, ag, xgs, pre)
            # drain the remaining tail pair-chains (slot 18 consumed 8 of
            # the 16 yields)
            for _ in tailg:
                pass

    nc.compile()
    nc.m = get_hw_module(nc.m)
    return nc


def kernel(hidden_states, attention_mask, w_attn, b_attn, w_proj, b_proj):
    global LAST_RESULTS
    hidden_states = np.asarray(hidden_states, dtype=np.float32)
    attention_mask = np.asarray(attention_mask, dtype=np.float32)
    w_attn = np.asarray(w_attn, dtype=np.float32)
    b_attn = np.asarray(b_attn, dtype=np.float32)
    w_proj = np.asarray(w_proj, dtype=np.float32)
    b_proj = np.asarray(b_proj, dtype=np.float32)

    use_mask = bool((attention_mask != 1.0).any())
    key = ("prog", use_mask)
    if key not in _PROG_CACHE:
        _PROG_CACHE[key] = _build_program(use_mask)
    nc = _PROG_CACHE[key]

    in_maps = []
    for c in range(N_CORES):
        b, g = divmod(c, 4)
        # xt: [D, S] -> [128, (tb, kt, tok512)]
        xt = hidden_states[b].T.astype(NPF16)                  # [D, S]
        xt_p = np.ascontiguousarray(
            xt.reshape(KT, 128, QB, 512).transpose(1, 2, 0, 3)
        ).reshape(128, QB * KT * 512)
        # wqk: [D, 1024] -> [128, (kt, f, c128)]  (kt-major: the tb0 QK
        # rounds consume one kt piece across all 8 f-chunks per round)
        wq = w_attn[:, g * FL:(g + 1) * FL]
        wk = w_attn[:, D + g * FL:D + (g + 1) * FL]
        wqk = np.concatenate([wq, wk], axis=1).astype(NPF16)   # [D, 1024]
        wqk_p = np.ascontiguousarray(
            wqk.reshape(KT, 128, 8, 128).transpose(1, 0, 2, 3)
        ).reshape(128, 8 * FW)
        # wv: [D, FL] -> [128, (kt, FL)]
        wv = w_attn[:, 2 * D + g * FL:2 * D + (g + 1) * FL].astype(NPF16)
        wv_p = np.ascontiguousarray(
            wv.reshape(KT, 128, FL).transpose(1, 0, 2)
        ).reshape(128, KT * FL)
        wp = np.ascontiguousarray(w_proj[g * FL:(g + 1) * FL, :]).astype(NPF16)
        bq = b_attn[g * FL:(g + 1) * FL]
        bk = b_attn[D + g * FL:D + (g + 1) * FL]
        bqk = np.ascontiguousarray(
            np.concatenate([bq, bk]).reshape(8, 128).T).astype(np.float32)
        kb = np.ascontiguousarray(
            attention_mask[b].reshape(KT, 128).T).astype(np.float32)
        in_maps.append({
            "xt": np.ascontiguousarray(xt_p),
            "wqk": np.ascontiguousarray(wqk_p),
            "wv": np.ascontiguousarray(wv_p),
            "wp": wp,
            "bqk": bqk,
            "kb": kb,
        })

    if not os.environ.get("KERNEL_ALLOW_TRACE"):
        os.environ["BASS_NEVER_TRACE"] = "1"
    try:
        res = bass_utils.run_bass_kernel_spmd(nc, in_maps,
                                              list(range(N_CORES)))
    except Exception:
        # Transient NRT failures can leave the axon device wedged; reset it
        # once and retry.
        try:
            import ctypes

            import jax

            jax.devices()
            _lib = ctypes.CDLL("/opt/axon/libaxon_pjrt.so")
            _lib.axon_reset.restype = ctypes.c_int64
            _lib.axon_reset()
        except Exception:
            pass
        res = bass_utils.run_bass_kernel_spmd(nc, in_maps,
                                              list(range(N_CORES)))
    LAST_RESULTS = res

    # host reduce: sum the 4 head-group partials per batch, add biases.
    # V-bias contribution: rows of A sum to 1, so each core's O gains b_v
    # per row; through c_proj that's a constant row b_v @ w_proj_slice.
    out = np.zeros((B, S, D), dtype=np.float32)
    for c in range(N_CORES):
        b, g = divmod(c, 4)
        out[b] += res.results[c]["out"].astype(np.float32)
    bias_row = b_proj.astype(np.float64).copy()
    for g in range(4):
        bv = b_attn[2 * D + g * FL:2 * D + (g + 1) * FL].astype(np.float64)
        bias_row += bv @ w_proj[g * FL:(g + 1) * FL, :].astype(np.float64)
    out += bias_row.astype(np.float32)[None, None, :]
    return out

